# revision 47
# baseline (speedup 1.0000x reference)
"""CPC unsupervised criterion loss kernel for 8 Trainium2 NeuronCores.

Sharding: data-parallel over the nGt=8 batch axis, one sequence per core;
W and the otherEncoded negative pool replicated; the extIdx gather is local
per shard.

Per-core pipeline (v2 — fp16 data path, 4 SWDGE queues):
  - The pool is fp16 rows [8192, 256] (512 B): vs the fp32-accurate bf16
    hi/lo baseline this halves both the gather HBM bytes (7.6 MB/core) and
    the SWDGE RX descriptor count (elem_bytes/256 = 2 per index), so the
    gpsimd descriptor generation ALSO halves; 4 SWDGE queues pipeline
    desc-gen across all 8 gpsimd cores (2 sub-gathers of 114 descs fit in
    the 256-desc/engine carveout per queue).
  - The transposing dma_gather lands rows e-major: negT[e, h, i], the
    exact PE stationary layout (plane h = e-chunk h), no on-chip transpose.
  - locC = ct16 @ (64*W)^T on PE in fp16 (fp32 PSUM), stored fp16 e-major
    (scale 1/64) as lcg[mc][e, t*12+k]; scores come out x256 of the
    reference (ct is not pre-divided by dE), fixed in the exp/ln scales.
    fp16 keeps |values| ~1 so no fp16-subnormal/FTZ hazard.
  - negScore matmuls: per tau, 2 matmuls (e-chunks) accumulate [128 negs,
    12 k] into a per-group PSUM tile; a DVE copy reorders (tau,k)->(k,tau),
    PE-transposes to [(k,tau) x 128 negs], then max (accuracy) and
    sum(exp(s/256)) (logsumexp; |s/256| << 1 so no max subtraction).
  - Positive path: p2[t,e] = 64*locC on PE, DVE multiply with gts = gt/64
    and reduce -> posT = 256*posScore; combine per group right after its
    reductions land so the post-gather tail stays short.
  - fp16 end-to-end was validated on the reference inputs: 0 argmax flips
    (min pos-vs-maxneg gap 8.1e-5 vs ~1.4e-5 score error), loss err 2e-6.
"""

import os
import sys

import numpy as np

try:
    import concourse  # noqa: F401
except ImportError:
    sys.path.insert(0, "/opt/trn_rl_repo")

import ml_dtypes  # noqa: F401

import concourse.bacc as bacc
import concourse.bass as bass
import concourse.mybir as mybir
import concourse.tile as tile
from concourse import library_config
from concourse.bass_utils import run_bass_kernel_spmd

F32 = mybir.dt.float32
F16 = mybir.dt.float16
I16 = mybir.dt.int16

N_PREDICTS = 12
DIM = 256
NEG = 128
N_GT = 8
SEQ_LEN = 128
POOL = 8192
WIN = SEQ_LEN - N_PREDICTS  # 116

NCORES = 8
TG = 32                                  # taus per reduce group
NGROUPS = 4
GCNT = [32, 32, 32, 20]                  # taus per group; sums to 116
SUBCNT = [4] * 24 + [5] * 4              # taus per sub-gather (7 rounds x 4 queues)
NSUB = len(SUBCNT)
SUBOFF = [sum(SUBCNT[:i]) for i in range(NSUB + 1)]   # tau prefix offsets
# tau -> (sub index, offset within sub)
TAU2SUB = []
for _si in range(NSUB):
    for _o in range(SUBCNT[_si]):
        TAU2SUB.append((_si, _o))
SBUF_SRC = False                         # gather from an SBUF-resident pool
ACT = mybir.ActivationFunctionType
ALU = mybir.AluOpType

_prog_cache = None


def _build_program():
    nc = bacc.Bacc("TRN2", target_bir_lowering=False, debug=False,
                   num_devices=NCORES, num_swdge_queues=4)

    pool16 = nc.declare_dram_parameter("pool16", [POOL, DIM], F16, isOutput=False)
    poolsb = nc.declare_dram_parameter("poolsb", [128, POOL * 2], F16, isOutput=False)
    wt = nc.declare_dram_parameter("wt", [128, N_PREDICTS * 2 * DIM], F16, isOutput=False)
    ct = nc.declare_dram_parameter("ct", [128, 2 * WIN], F16, isOutput=False)
    gt = nc.declare_dram_parameter("gt", [128, DIM], F32, isOutput=False)
    idx = nc.declare_dram_parameter("idx", [128, NEG * WIN // 16], I16, isOutput=False)
    ones = nc.declare_dram_parameter("ones", [128, 1], F32, isOutput=False)
    ident = nc.declare_dram_parameter("ident", [128, 128], F32, isOutput=False)
    out = nc.declare_dram_parameter("out", [1, 2 * N_PREDICTS], F32, isOutput=True)

    with tile.TileContext(nc) as tc:
        with (
            tc.tile_pool(name="constp", bufs=1) as constp,
            tc.tile_pool(name="sbp", bufs=1) as sbp,
            tc.tile_pool(name="negp", bufs=NSUB) as negp,
            tc.tile_pool(name="workp", bufs=2) as workp,
        ):
            # --- gpsimd library + gather inputs as early as possible ---
            lib = nc.gpsimd.load_library(library_config.mlp)
            idxs = constp.tile([128, NEG * WIN // 16], I16)
            IHALF = (NEG * WIN // 16) // 2
            nc.sync.dma_start(idxs[:, :IHALF], idx[:, :IHALF])
            nc.scalar.dma_start(idxs[:, IHALF:], idx[:, IHALF:])
            if SBUF_SRC:
                # pool resident in SBUF: row j at partition j%128, rank j//128
                poolt = constp.tile([128, POOL * 2], F16)
                nc.sync.dma_start(poolt[:, :POOL], poolsb[:, :POOL])
                nc.scalar.dma_start(poolt[:, POOL:], poolsb[:, POOL:])

            # transposing SWDGE gathers: negT[e, h, i] = pool[idx_i, e+128h].
            # fp16 rows are 512 B -> 114 RX descs per engine per 896-idx
            # sub-gather; two fit in the 256-desc carveout per queue, and
            # 4 queues let 8 desc-gens pipeline across the gpsimd cores.
            negts = []
            for i in range(NSUB):
                n = SUBCNT[i] * NEG
                t = negp.tile([128, 2, n], F16, tag="negT", name=f"negT{i}")
                ioff = SUBOFF[i] * NEG // 16
                if SBUF_SRC:
                    gi = nc.gpsimd.dma_gather(
                        t[:], poolt[:],
                        idxs[:, ioff:ioff + n // 16],
                        n, n, DIM, transpose=True, queue_num=i % 4,
                        sbuf_tokens_per_rank=128,
                        sbuf_free_dim_per_rank=2 * DIM,
                    )
                else:
                    gi = nc.gpsimd.dma_gather(
                        t[:], pool16[:],
                        idxs[:, ioff:ioff + n // 16],
                        n, n, DIM, transpose=True, queue_num=i % 4,
                    )
                bass._add_dep_helper(gi.ins, lib.ins, sync=False,
                                     reason="gpsimd lib before gather")
                negts.append(t)

            # --- constant loads ---
            ctile = constp.tile([128, 2 * WIN], F16)
            nc.sync.dma_start(ctile[:], ct[:])
            wtile = constp.tile([128, N_PREDICTS * 2 * DIM], F16)
            HW_ = N_PREDICTS * DIM
            nc.sync.dma_start(wtile[:, :HW_], wt[:, :HW_])
            nc.sync.dma_start(wtile[:, HW_:], wt[:, HW_:])
            gtile = constp.tile([128, DIM], F32)
            nc.scalar.dma_start(gtile[:], gt[:])
            onest = constp.tile([128, 1], F32)
            nc.scalar.dma_start(onest[:], ones[:])
            identt = constp.tile([128, 128], F32)
            nc.scalar.dma_start(identt[:], ident[:])

            # shifted copies of gt for the positive path:
            # gts[t, k*256+e] = gt[t+k+1, e] / 64
            gts = constp.tile([128, N_PREDICTS * DIM], F32)
            for k in range(N_PREDICTS):
                nc.scalar.dma_start(gts[:WIN, k * DIM:(k + 1) * DIM],
                                    gtile[k + 1:k + 1 + WIN, :])

            # --- locC, e-major fp16: lcg[mc][p, t*12+k] = locC[k, t, 128mc+p] ---
            lcg = [constp.tile([128, WIN * N_PREDICTS], F16,
                               tag=f"lcg{c}", name=f"lcg{c}")
                   for c in range(2)]
            posT = sbp.tile([WIN, N_PREDICTS], F32)

            with tc.tile_pool(name="ps_lc", bufs=2, space="PSUM") as ps_lc:
                for mc in range(2):
                    for k in range(N_PREDICTS):
                        p1 = ps_lc.tile([128, WIN], F32, tag="l1")
                        for dc in range(2):
                            nc.tensor.matmul(
                                p1[:, :],
                                wtile[:, (k * 2 + dc) * DIM + mc * 128:
                                      (k * 2 + dc) * DIM + mc * 128 + 128],
                                ctile[:, dc * WIN:(dc + 1) * WIN],
                                start=(dc == 0), stop=(dc == 1),
                            )
                        o_ap = lcg[mc][:].rearrange("p (t x) -> p t x",
                                                    x=N_PREDICTS)[:, :, k]
                        nc.scalar.activation(o_ap, p1[:, :], ACT.Copy,
                                             scale=1.0 / 64.0)

            pools2 = (
                tc.tile_pool(name="ps_pos", bufs=2, space="PSUM"),
                tc.tile_pool(name="ps_sc", bufs=2, space="PSUM"),
                tc.tile_pool(name="ps_tr", bufs=2, space="PSUM"),
                tc.tile_pool(name="ps_fin", bufs=1, space="PSUM"),
            )
            with pools2[0] as ps_pos, pools2[1] as ps_sc, \
                    pools2[2] as ps_tr, pools2[3] as ps_fin:
                # --- positive path (early; overlaps the gather window) ---
                for k in range(N_PREDICTS):
                    p2 = ps_pos.tile([WIN, DIM], F32, tag="l2", name="l2")
                    for dc in range(2):
                        nc.tensor.matmul(
                            p2[:, :],
                            ctile[:, dc * WIN:(dc + 1) * WIN],
                            wtile[:, (k * 2 + dc) * DIM:(k * 2 + dc + 1) * DIM],
                            start=(dc == 0), stop=(dc == 1),
                        )
                    scr = workp.tile([WIN, DIM], F32, tag="scr", name="scr")
                    nc.vector.tensor_tensor(out=scr[:, :], in0=p2[:, :],
                                            in1=gts[:WIN, k * DIM:(k + 1) * DIM],
                                            op=ALU.mult)
                    kp = 3 * (k % 4) + k // 4
                    nc.vector.tensor_reduce(out=posT[:, kp:kp + 1],
                                            in_=scr[:, :],
                                            axis=mybir.AxisListType.X, op=ALU.add)

                # pos32[tau, g*12+kp] = 256*posScore[k, 32g+tau], where the
                # on-device predictor order is kp = 3*(k%4) + k//4 (so every
                # bridge DMA stays <= 3 AP dims); the host un-permutes.
                pos32 = sbp.tile([TG, NGROUPS * N_PREDICTS], F32)
                for g in range(NGROUPS):
                    nc.sync.dma_start(pos32[:GCNT[g], g * 12:(g + 1) * 12],
                                      posT[TG * g:TG * g + GCNT[g], :])
                posTr = sbp.tile([TG, NGROUPS * N_PREDICTS], F32)
                nc.scalar.activation(posTr[:, :], pos32[:, :], ACT.Copy,
                                     scale=1.0 / 256.0)

                expP = sbp.tile([TG, NGROUPS * N_PREDICTS], F32)
                nc.scalar.activation(expP[:, :], pos32[:, :], ACT.Exp,
                                     scale=1.0 / 256.0)

                # MS128[r, q*12 + (g*3+j)]: q=0 max, q=1 sum(exp); r=kap*32+tau
                MS128 = sbp.tile([128, 2 * N_PREDICTS], F32)
                # MS32[tau, q*48 + g*12 + kp], kp = 3*kap + j
                MS32 = sbp.tile([TG, 2 * NGROUPS * N_PREDICTS], F32)
                ms128r = MS128[:].rearrange("p (q c) -> p q c", q=2)
                ms32r = MS32[:].rearrange("p (q g kp) -> p q g kp",
                                          q=2, g=NGROUPS)
                lnwarm = sbp.tile([1, 1], F32)
                nc.vector.memset(lnwarm[:, :], 1.0)

                def reduce_group(g, ssb):
                    last = g == NGROUPS - 1
                    for j in range(3):
                        trp = ps_tr.tile([128, 128], F32, tag="trp", name="trp")
                        nc.tensor.transpose(trp[:, :],
                                            ssb[:, j * 128:(j + 1) * 128],
                                            identt[:, :])
                        c2 = g * 3 + j
                        nc.vector.tensor_reduce(out=MS128[:, c2:c2 + 1],
                                                in_=trp[:, :],
                                                axis=mybir.AxisListType.X,
                                                op=ALU.max)
                        esc = workp.tile([128, 128], F32, tag="esc", name="esc")
                        nc.scalar.activation(esc[:, :], trp[:, :], ACT.Exp,
                                             scale=1.0 / 256.0)
                        nc.vector.tensor_reduce(out=MS128[:, 12 + c2:13 + c2],
                                                in_=esc[:, :],
                                                axis=mybir.AxisListType.X,
                                                op=ALU.add)
                    if last:
                        # preload the Ln activation table while the bridges
                        # run so the final logsumexp is not table-load bound
                        nc.scalar.activation(lnwarm[:, :], lnwarm[:, :], ACT.Ln)
                    # bridge this group's columns r-layout -> tau-layout;
                    # one DMA per kap moves max+sumexp, split over two rings
                    # (the last group's go all on sync, parallel to the warm)
                    rows = GCNT[g]
                    for kap in range(4):
                        eng = nc.sync if (kap < 2 or last) else nc.scalar
                        eng.dma_start(
                            ms32r[:rows, :, g, 3 * kap:3 * kap + 3],
                            ms128r[kap * TG:kap * TG + rows, :,
                                   3 * g:3 * g + 3])

                # --- main loop over groups of up to 32 taus ---
                pending = []
                for g in range(NGROUPS):
                    cnt = GCNT[g]
                    psg = ps_sc.tile([128, TG * N_PREDICTS], F32, tag="ps",
                                     name="ps")
                    for tau in range(cnt):
                        t = TG * g + tau
                        ci, o = TAU2SUB[t]
                        negT = negts[ci]
                        ms = psg[:, tau * 12:tau * 12 + 12]
                        lcs = slice(t * 12, t * 12 + 12)
                        sl = slice(o * NEG, (o + 1) * NEG)
                        nc.tensor.matmul(ms, negT[:, 0, sl], lcg[0][:, lcs],
                                         start=True, stop=False)
                        nc.tensor.matmul(ms, negT[:, 1, sl], lcg[1][:, lcs],
                                         start=False, stop=True)

                    # (tau,k)->(k,tau) reorder: ssb[p, k*32+tau]
                    ssb = workp.tile([128, N_PREDICTS * TG], F32, tag="ssb",
                                     bufs=4)
                    o_ap = ssb[:].rearrange("p (k t) -> p t k", k=N_PREDICTS)
                    ip = psg[:].rearrange("p (t x) -> p t x", x=N_PREDICTS)
                    nc.vector.tensor_copy(o_ap[:, 0:cnt, :], ip[:, 0:cnt, :])

                    # lag the reductions by one group early on (keeps PE
                    # dense); run immediately for the last two groups so the
                    # post-gather tail only contains the final group's chain
                    pending.append((g, ssb))
                    if g >= NGROUPS - 2:
                        while pending:
                            reduce_group(*pending.pop(0))
                    elif len(pending) > 1:
                        reduce_group(*pending.pop(0))

                # --- final combine (single Ln -> one act-table switch) ---
                tmp1 = sbp.tile([TG, NGROUPS * N_PREDICTS], F32)
                nc.vector.tensor_tensor(out=tmp1[:, :], in0=expP[:, :],
                                        in1=MS32[:, 48:], op=ALU.add)
                tmp2 = sbp.tile([TG, NGROUPS * N_PREDICTS], F32)
                nc.scalar.activation(tmp2[:, :], tmp1[:, :], ACT.Ln)
                comb = sbp.tile([TG, NGROUPS * N_PREDICTS * 2], F32)
                c_ap = comb[:].rearrange("p (g k q) -> p q g k", q=2, g=NGROUPS)
                t2 = tmp2[:].rearrange("p (g k) -> p g k", g=NGROUPS)
                pr = posTr[:].rearrange("p (g k) -> p g k", g=NGROUPS)
                p2_ = pos32[:].rearrange("p (g k) -> p g k", g=NGROUPS)
                m2 = MS32[:, :48].rearrange("p (g k) -> p g k", g=NGROUPS)
                nc.vector.tensor_tensor(out=c_ap[:, 0], in0=t2, in1=pr,
                                        op=ALU.subtract)
                nc.vector.tensor_tensor(out=c_ap[:, 1], in0=p2_, in1=m2,
                                        op=ALU.is_ge)

                # --- sum over taus via ones-matmul, scale, store ---
                fin = ps_fin.tile([1, 2 * N_PREDICTS], F32, tag="fin")
                for g in range(NGROUPS):
                    nc.tensor.matmul(fin[:, :], onest[:GCNT[g], :],
                                     comb[:GCNT[g], g * 24:(g + 1) * 24],
                                     start=(g == 0), stop=(g == NGROUPS - 1))
                outsb = sbp.tile([1, 2 * N_PREDICTS], F32)
                f_ap = fin[:].rearrange("p (k q) -> p q k", q=2)
                os_ap = outsb[:].rearrange("p (k q) -> p q k", q=2)
                nc.scalar.activation(os_ap[:, 0], f_ap[:, 0], ACT.Copy,
                                     scale=1.0 / WIN)
                nc.scalar.activation(os_ap[:, 1], f_ap[:, 1], ACT.Copy,
                                     scale=1.0 / (N_GT * WIN))
                nc.sync.dma_start(out[:], outsb[:])

    nc.compile()
    return nc


def _host_prep(cFeature, gtPredictions, otherEncoded, W, extIdx):
    """Build the 8 per-core input maps."""
    pool16 = np.ascontiguousarray(
        np.asarray(otherEncoded, dtype=np.float32).astype(np.float16))
    # poolsb[p, r*256+e] = pool[r*128+p, e]
    poolsb = np.ascontiguousarray(
        pool16.reshape(POOL // 128, 128, DIM).transpose(1, 0, 2)
        .reshape(128, POOL * 2))

    W64 = np.asarray(W, dtype=np.float32) * 64.0
    # wt[p, (k*2+dc)*256 + e] = 64*W[k, e, 128dc+p]
    wt_np = np.ascontiguousarray(
        W64.transpose(0, 2, 1).reshape(N_PREDICTS, 2, 128, DIM)
        .transpose(2, 0, 1, 3).reshape(128, N_PREDICTS * 2 * DIM)
        .astype(np.float16))

    ones_np = np.ones((128, 1), dtype=np.float32)
    ident_np = np.eye(128, dtype=np.float32)

    ext = np.asarray(extIdx).reshape(N_GT, NEG, WIN)

    in_maps = []
    for b in range(N_GT):
        cb = np.asarray(cFeature[b, :WIN], dtype=np.float32)  # [116, 256]
        # ct[p, dc*116+t] = c[t, 128dc+p]
        ct_np = np.ascontiguousarray(
            cb.T.reshape(2, 128, WIN).transpose(1, 0, 2).reshape(128, 2 * WIN)
            .astype(np.float16))
        gt_np = np.ascontiguousarray(
            np.asarray(gtPredictions[b], dtype=np.float32) / 64.0)
        flat = np.ascontiguousarray(ext[b].T).reshape(-1)  # i = t*128 + n
        idx_np = np.ascontiguousarray(
            np.tile(flat.reshape(-1, 16).T, (8, 1))).astype(np.int16)
        in_maps.append({
            "pool16": pool16,
            "poolsb": poolsb,
            "wt": wt_np,
            "ct": ct_np,
            "gt": gt_np,
            "idx": idx_np,
            "ones": ones_np,
            "ident": ident_np,
        })
    return in_maps


def kernel(cFeature, gtPredictions, otherEncoded, W, extIdx):
    global _prog_cache
    if _prog_cache is None:
        _prog_cache = _build_program()
    nc = _prog_cache
    in_maps = _host_prep(cFeature, gtPredictions, otherEncoded, W, extIdx)
    res = run_bass_kernel_spmd(nc, in_maps, list(range(NCORES)))
    losses = np.zeros(N_PREDICTS * N_GT, dtype=np.float32)
    acc = np.zeros(N_PREDICTS * N_GT, dtype=np.float32)
    # device row kp = 3*(k%4) + k//4  ->  true predictor k = 4*(kp%3) + kp//3
    kperm = np.array([4 * (kp % 3) + kp // 3 for kp in range(N_PREDICTS)])
    for b in range(N_GT):
        o = res.results[b]["out"].reshape(N_PREDICTS, 2)
        losses[kperm * N_GT + b] = o[:, 0]
        acc[kperm * N_GT + b] = o[:, 1]
    return losses, acc


if __name__ == "__main__":
    sys.path.insert(0, os.path.dirname(os.path.abspath(__file__)))
    import reference

    inputs = reference.setup_inputs()
    inputs = {k: np.asarray(v) for k, v in inputs.items()}
    got_losses, got_acc = kernel(**inputs)
    print("losses:", got_losses[:8])
    print("acc:", got_acc[:8])
